# revision 2
# baseline (speedup 1.0000x reference)
"""Trainium2 Bass kernel for nn_LowRankRotatedSpaceIntervention.

Reference computation (B=8192, D=4096, r=512, k=128):
    sel  = subspaces[0]                  # shared index set (fast path)
    diff = (source - base) @ W           # [B, r]
    out  = base + diff[:, sel] @ W[:, sel].T

Only the selected k=128 columns of W matter:
    out = base + ((source - base) @ W_sel) @ W_sel.T,  W_sel = W[:, sel]

Sharding: data-parallel over batch across 8 NeuronCores; W_sel (2 MiB)
replicated. Host precomputes W_sel and W_sel.T (cheap) from subspaces[0].

Device kernel per core (batch shard 1024 rows, 8 blocks of 128):
    load base/source block [128, 4096] f32
    D  = source - base                    (DVE, output bf16)
    Dt = PE-transpose of D in [128,128] chunks (bf16, via identity matmul)
    T^T[k,128] = sum_j W_sel_chunk_j.T @ Dt_chunk_j   (32 bf16 matmuls, psum)
    out_block = base + (T^T).T @ W_selT   (8 fp32 matmuls N=512 + DVE add)
    store out_block

The correction term has rms ~0.25 vs base ~1.0, so bf16 rounding on the
first matmul contributes ~1e-3 absolute error on the output; the second
matmul and the final add are fp32.
"""

import os
import numpy as np
import ml_dtypes

import concourse.bass as bass
import concourse.tile as tile
from concourse import bacc, masks, mybir
from concourse.bass_utils import run_bass_kernel_spmd

N_CORES = 8
B_FULL = 8192
D = 4096
K = 128
BS = B_FULL // N_CORES  # 1024 rows per core
NB = BS // 128          # 8 blocks of 128 rows
NCH = D // 128          # 32 contraction chunks of 128

F32 = mybir.dt.float32
BF16 = mybir.dt.bfloat16


def _build(mm1_dtype="bf16", mm2_f32r=False):
    nc = bacc.Bacc("TRN2", target_bir_lowering=False, debug=False)

    base_d = nc.dram_tensor("base", [BS, D], F32, kind="ExternalInput").ap()
    src_d = nc.dram_tensor("source", [BS, D], F32, kind="ExternalInput").ap()
    w1_dt = BF16 if mm1_dtype == "bf16" else F32
    # w1: chunk-major W_sel: w1[p, 128*j + k] = W_sel[128*j + p, k]
    w1_d = nc.dram_tensor("w1", [128, D], w1_dt, kind="ExternalInput").ap()
    # w2: W_sel.T  (k on partitions)
    w2_d = nc.dram_tensor("w2", [K, D], F32, kind="ExternalInput").ap()
    out_d = nc.dram_tensor("out", [BS, D], F32, kind="ExternalOutput").ap()

    with tile.TileContext(nc) as tc:
        with (
            tc.tile_pool(name="wpool", bufs=1) as wpool,
            tc.tile_pool(name="ipool", bufs=1) as ipool,
            tc.tile_pool(name="bpool", bufs=3) as bpool,
            tc.tile_pool(name="spool", bufs=2) as spool,
            tc.tile_pool(name="dpool", bufs=2) as dpool,
            tc.tile_pool(name="dtpool", bufs=2) as dtpool,
            tc.tile_pool(name="ttpool", bufs=2) as ttpool,
            tc.tile_pool(name="opool", bufs=2) as opool,
            tc.tile_pool(name="ptr", bufs=2, space="PSUM") as ptrpool,
            tc.tile_pool(name="pT", bufs=2, space="PSUM") as pTpool,
            tc.tile_pool(name="p2", bufs=3, space="PSUM") as p2pool,
        ):
            w1_sb = wpool.tile([128, D], w1_dt, tag="w1")
            nc.sync.dma_start(w1_sb[:], w1_d[:])
            w2_sb = wpool.tile([K, D], F32, tag="w2")
            nc.sync.dma_start(w2_sb[:], w2_d[:])
            ident = ipool.tile([128, 128], w1_dt, tag="ident")
            masks.make_identity(nc, ident[:])

            # transposes per psum bank: bf16 bank holds 8 chunks, f32 bank 4
            per_bank = 8 if w1_dt == BF16 else 4
            bank_free = 128 * per_bank

            # process blocks in pairs so mm1 runs with N=256 moving dim
            for ip in range(NB // 2):
                bts, dbs = [], []
                # Dt for the pair, block-major: dtt[p, D*par + 128*j + b]
                dtt = dtpool.tile([128, 2 * D], w1_dt, tag="dtt")
                for par in range(2):
                    i = 2 * ip + par
                    bt = bpool.tile([128, D], F32, tag="bt")
                    nc.sync.dma_start(bt[:], base_d[128 * i : 128 * (i + 1), :])
                    st = spool.tile([128, D], F32, tag="st")
                    nc.sync.dma_start(st[:], src_d[128 * i : 128 * (i + 1), :])
                    bts.append(bt)

                    if mm1_dtype == "bf16":
                        db = dpool.tile([128, D], BF16, tag="db")
                        nc.vector.tensor_sub(db[:], st[:], bt[:])
                    else:
                        db = st  # subtract in place, keep f32
                        nc.vector.tensor_sub(db[:], st[:], bt[:])

                    for g in range(NCH // per_bank):
                        ps = ptrpool.tile([128, bank_free], w1_dt, tag="ps")
                        for q in range(per_bank):
                            j = per_bank * g + q
                            nc.tensor.transpose(
                                ps[:, 128 * q : 128 * (q + 1)],
                                db[:, 128 * j : 128 * (j + 1)],
                                ident[:],
                            )
                        nc.scalar.copy(
                            dtt[:, D * par + bank_free * g : D * par + bank_free * (g + 1)],
                            ps[:],
                        )

                # mm1: T^T for both blocks, N=256 via 3D AP (par, b) over dtt
                dt3 = dtt[:].rearrange("p (par j b) -> p j par b", par=2, b=128)
                pt = pTpool.tile([K, 256], F32, tag="pt")
                for j in range(NCH):
                    nc.tensor.matmul(
                        pt[:],
                        w1_sb[:, 128 * j : 128 * (j + 1)],
                        dt3[:, j],
                        start=(j == 0),
                        stop=(j == NCH - 1),
                    )
                ttt = ttpool.tile([K, 256], F32, tag="ttt")
                nc.vector.tensor_copy(ttt[:], pt[:])

                for par in range(2):
                    i = 2 * ip + par
                    bt = bts[par]
                    ot = opool.tile([128, D], F32, tag="ot")
                    for dj in range(D // 512):
                        p2t = p2pool.tile([128, 512], F32, tag="p2t")
                        lhs = ttt[:, 128 * par : 128 * (par + 1)]
                        rhs = w2_sb[:, 512 * dj : 512 * (dj + 1)]
                        if mm2_f32r:
                            lhs = lhs.bitcast(mybir.dt.float32r)
                            rhs = rhs.bitcast(mybir.dt.float32r)
                        nc.tensor.matmul(p2t[:], lhs, rhs, start=True, stop=True)
                        nc.vector.tensor_add(
                            ot[:, 512 * dj : 512 * (dj + 1)],
                            bt[:, 512 * dj : 512 * (dj + 1)],
                            p2t[:],
                        )
                    nc.sync.dma_start(out_d[128 * i : 128 * (i + 1), :], ot[:])

    nc.compile()
    return nc


_NC_CACHE = {}


def _get_nc(mm1_dtype, mm2_f32r):
    key = (mm1_dtype, mm2_f32r)
    if key not in _NC_CACHE:
        _NC_CACHE[key] = _build(mm1_dtype, mm2_f32r)
    return _NC_CACHE[key]


def make_in_maps(inputs, mm1_dtype="bf16"):
    base = np.ascontiguousarray(np.asarray(inputs["base"], dtype=np.float32))
    source = np.ascontiguousarray(np.asarray(inputs["source"], dtype=np.float32))
    subspaces = np.asarray(inputs["subspaces"])
    W = np.asarray(inputs["W"], dtype=np.float32)
    assert base.shape == (B_FULL, D) and source.shape == (B_FULL, D)

    sel = np.asarray(subspaces[0]).astype(np.int64)  # shared index set
    W_sel = np.ascontiguousarray(W[:, sel])          # [D, K] f32
    # chunk-major layout: w1[p, 128*j + k] = W_sel[128*j + p, k]
    w1 = np.ascontiguousarray(
        W_sel.reshape(NCH, 128, K).transpose(1, 0, 2).reshape(128, D)
    )
    if mm1_dtype == "bf16":
        w1 = w1.astype(ml_dtypes.bfloat16)
    w2 = np.ascontiguousarray(W_sel.T)               # [K, D] f32

    in_maps = []
    for c in range(N_CORES):
        in_maps.append(
            {
                "base": np.ascontiguousarray(base[c * BS : (c + 1) * BS]),
                "source": np.ascontiguousarray(source[c * BS : (c + 1) * BS]),
                "w1": w1,
                "w2": w2,
            }
        )
    return in_maps


def run(inputs, trace=False, mm1_dtype="bf16", mm2_f32r=False):
    nc = _get_nc(mm1_dtype, mm2_f32r)
    in_maps = make_in_maps(inputs, mm1_dtype)
    res = run_bass_kernel_spmd(nc, in_maps, list(range(N_CORES)), trace=trace)
    out = np.concatenate([r["out"] for r in res.results], axis=0)
    return out, res


def kernel(**inputs) -> np.ndarray:
    mm1_dtype = os.environ.get("LRI_MM1", "bf16")
    mm2_f32r = os.environ.get("LRI_MM2_F32R", "0") == "1"
    out, _ = run(inputs, trace=False, mm1_dtype=mm1_dtype, mm2_f32r=mm2_f32r)
    return out


# revision 4
# speedup vs baseline: 1.0600x; 1.0600x over previous
"""Trainium2 Bass kernel for nn_LowRankRotatedSpaceIntervention.

Reference computation (B=8192, D=4096, r=512, k=128):
    sel  = subspaces[0]                  # shared index set (fast path)
    diff = (source - base) @ W           # [B, r]
    out  = base + diff[:, sel] @ W[:, sel].T

Only the selected k=128 columns of W matter:
    out = base + ((source - base) @ W_sel) @ W_sel.T,  W_sel = W[:, sel]

Sharding: data-parallel over batch across 8 NeuronCores; W_sel (2 MiB)
replicated. Host precomputes W_sel and W_sel.T (cheap) from subspaces[0].

Device kernel per core (batch shard 1024 rows, 8 blocks of 128):
    load base/source block [128, 4096] f32
    D  = source - base                    (DVE, output bf16)
    Dt = PE-transpose of D in [128,128] chunks (bf16, via identity matmul)
    T^T[k,128] = sum_j W_sel_chunk_j.T @ Dt_chunk_j   (32 bf16 matmuls, psum)
    out_block = base + (T^T).T @ W_selT   (8 fp32 matmuls N=512 + DVE add)
    store out_block

The correction term has rms ~0.25 vs base ~1.0, so bf16 rounding on the
first matmul contributes ~1e-3 absolute error on the output; the second
matmul and the final add are fp32.
"""

import os
import numpy as np
import ml_dtypes

import concourse.bass as bass
import concourse.tile as tile
from concourse import bacc, masks, mybir
from concourse.bass_utils import run_bass_kernel_spmd

N_CORES = 8
B_FULL = 8192
D = 4096
K = 128
BS = B_FULL // N_CORES  # 1024 rows per core
NB = BS // 128          # 8 blocks of 128 rows
NCH = D // 128          # 32 contraction chunks of 128

F32 = mybir.dt.float32
BF16 = mybir.dt.bfloat16


def _build(mm1_dtype="bf16", mm2_f32r=False):
    nc = bacc.Bacc("TRN2", target_bir_lowering=False, debug=False)

    base_d = nc.dram_tensor("base", [BS, D], F32, kind="ExternalInput").ap()
    src_d = nc.dram_tensor("source", [BS, D], F32, kind="ExternalInput").ap()
    w1_dt = BF16 if mm1_dtype == "bf16" else F32
    # fp32r is bit-identical to fp32; declaring the whole w2/ttt path as
    # fp32r satisfies the BIR verifier's "rounded to FP32r" producer rule.
    w2_dt = mybir.dt.float32r if mm2_f32r else F32
    # w1: chunk-major W_sel: w1[p, 128*j + k] = W_sel[128*j + p, k]
    w1_d = nc.dram_tensor("w1", [128, D], w1_dt, kind="ExternalInput").ap()
    # w2: W_sel.T  (k on partitions)
    w2_d = nc.dram_tensor("w2", [K, D], w2_dt, kind="ExternalInput").ap()
    out_d = nc.dram_tensor("out", [BS, D], F32, kind="ExternalOutput").ap()

    with tile.TileContext(nc) as tc:
        with (
            tc.tile_pool(name="wpool", bufs=1) as wpool,
            tc.tile_pool(name="ipool", bufs=1) as ipool,
            tc.tile_pool(name="bpool", bufs=3) as bpool,
            tc.tile_pool(name="spool", bufs=2) as spool,
            tc.tile_pool(name="dpool", bufs=2) as dpool,
            tc.tile_pool(name="dtpool", bufs=2) as dtpool,
            tc.tile_pool(name="ttpool", bufs=2) as ttpool,
            tc.tile_pool(name="opool", bufs=2) as opool,
            tc.tile_pool(name="ptr", bufs=2, space="PSUM") as ptrpool,
            tc.tile_pool(name="pT", bufs=2, space="PSUM") as pTpool,
            tc.tile_pool(name="p2", bufs=4, space="PSUM") as p2pool,
        ):
            w1_sb = wpool.tile([128, D], w1_dt, tag="w1")
            nc.sync.dma_start(w1_sb[:], w1_d[:])
            w2_sb = wpool.tile([K, D], w2_dt, tag="w2")
            nc.sync.dma_start(w2_sb[:], w2_d[:])
            ident = ipool.tile([128, 128], w1_dt, tag="ident")
            masks.make_identity(nc, ident[:])

            # transposes per psum bank: bf16 bank holds 8 chunks, f32 bank 4
            per_bank = 8 if w1_dt == BF16 else 4
            bank_free = 128 * per_bank

            # process blocks in pairs so mm1 runs with N=256 moving dim
            for ip in range(NB // 2):
                bts, dbs = [], []
                # Dt for the pair, block-major: dtt[p, D*par + 128*j + b]
                dtt = dtpool.tile([128, 2 * D], w1_dt, tag="dtt")
                for par in range(2):
                    i = 2 * ip + par
                    bt = bpool.tile([128, D], F32, tag="bt")
                    nc.sync.dma_start(bt[:], base_d[128 * i : 128 * (i + 1), :])
                    st = spool.tile([128, D], F32, tag="st")
                    nc.sync.dma_start(st[:], src_d[128 * i : 128 * (i + 1), :])
                    bts.append(bt)

                    if mm1_dtype == "bf16":
                        db = dpool.tile([128, D], BF16, tag="db")
                        nc.vector.tensor_sub(db[:], st[:], bt[:])
                    else:
                        db = st  # subtract in place, keep f32
                        nc.vector.tensor_sub(db[:], st[:], bt[:])

                    for g in range(NCH // per_bank):
                        ps = ptrpool.tile([128, bank_free], w1_dt, tag="ps")
                        for q in range(per_bank):
                            j = per_bank * g + q
                            nc.tensor.transpose(
                                ps[:, 128 * q : 128 * (q + 1)],
                                db[:, 128 * j : 128 * (j + 1)],
                                ident[:],
                            )
                        nc.scalar.copy(
                            dtt[:, D * par + bank_free * g : D * par + bank_free * (g + 1)],
                            ps[:],
                        )

                # mm1: T^T for both blocks, N=256 via 3D AP (par, b) over dtt
                dt3 = dtt[:].rearrange("p (par j b) -> p j par b", par=2, b=128)
                pt = pTpool.tile([K, 256], F32, tag="pt")
                for j in range(NCH):
                    nc.tensor.matmul(
                        pt[:],
                        w1_sb[:, 128 * j : 128 * (j + 1)],
                        dt3[:, j],
                        start=(j == 0),
                        stop=(j == NCH - 1),
                    )
                ttt = ttpool.tile([K, 256], w2_dt, tag="ttt")
                nc.vector.tensor_copy(ttt[:], pt[:])

                for par in range(2):
                    i = 2 * ip + par
                    bt = bts[par]
                    ot = opool.tile([128, D], F32, tag="ot")
                    for dj in range(D // 512):
                        p2t = p2pool.tile([128, 512], F32, tag="p2t")
                        lhs = ttt[:, 128 * par : 128 * (par + 1)]
                        rhs = w2_sb[:, 512 * dj : 512 * (dj + 1)]
                        nc.tensor.matmul(p2t[:], lhs, rhs, start=True, stop=True)
                        nc.vector.tensor_add(
                            ot[:, 512 * dj : 512 * (dj + 1)],
                            bt[:, 512 * dj : 512 * (dj + 1)],
                            p2t[:],
                        )
                    nc.sync.dma_start(out_d[128 * i : 128 * (i + 1), :], ot[:])

    nc.compile()
    return nc


_NC_CACHE = {}


def _get_nc(mm1_dtype, mm2_f32r):
    key = (mm1_dtype, mm2_f32r)
    if key not in _NC_CACHE:
        _NC_CACHE[key] = _build(mm1_dtype, mm2_f32r)
    return _NC_CACHE[key]


def make_in_maps(inputs, mm1_dtype="bf16"):
    base = np.ascontiguousarray(np.asarray(inputs["base"], dtype=np.float32))
    source = np.ascontiguousarray(np.asarray(inputs["source"], dtype=np.float32))
    subspaces = np.asarray(inputs["subspaces"])
    W = np.asarray(inputs["W"], dtype=np.float32)
    assert base.shape == (B_FULL, D) and source.shape == (B_FULL, D)

    sel = np.asarray(subspaces[0]).astype(np.int64)  # shared index set
    W_sel = np.ascontiguousarray(W[:, sel])          # [D, K] f32
    # chunk-major layout: w1[p, 128*j + k] = W_sel[128*j + p, k]
    w1 = np.ascontiguousarray(
        W_sel.reshape(NCH, 128, K).transpose(1, 0, 2).reshape(128, D)
    )
    if mm1_dtype == "bf16":
        w1 = w1.astype(ml_dtypes.bfloat16)
    w2 = np.ascontiguousarray(W_sel.T)               # [K, D] f32

    in_maps = []
    for c in range(N_CORES):
        in_maps.append(
            {
                "base": np.ascontiguousarray(base[c * BS : (c + 1) * BS]),
                "source": np.ascontiguousarray(source[c * BS : (c + 1) * BS]),
                "w1": w1,
                "w2": w2,
            }
        )
    return in_maps


def run(inputs, trace=False, mm1_dtype="bf16", mm2_f32r=False):
    nc = _get_nc(mm1_dtype, mm2_f32r)
    in_maps = make_in_maps(inputs, mm1_dtype)
    res = run_bass_kernel_spmd(nc, in_maps, list(range(N_CORES)), trace=trace)
    out = np.concatenate([r["out"] for r in res.results], axis=0)
    return out, res


def kernel(**inputs) -> np.ndarray:
    mm1_dtype = os.environ.get("LRI_MM1", "bf16")
    mm2_f32r = os.environ.get("LRI_MM2_F32R", "0") == "1"
    out, _ = run(inputs, trace=False, mm1_dtype=mm1_dtype, mm2_f32r=mm2_f32r)
    return out


# revision 7
# speedup vs baseline: 1.1905x; 1.1231x over previous
"""Trainium2 Bass kernel for nn_LowRankRotatedSpaceIntervention.

Reference computation (B=8192, D=4096, r=512, k=128):
    sel  = subspaces[0]                  # shared index set (fast path)
    diff = (source - base) @ W           # [B, r]
    out  = base + diff[:, sel] @ W[:, sel].T

Only the selected k=128 columns of W matter:
    out = base + ((source - base) @ W_sel) @ W_sel.T,  W_sel = W[:, sel]

Sharding: data-parallel over batch across 8 NeuronCores; W_sel (2 MiB)
replicated. Host precomputes W_sel and W_sel.T (cheap) from subspaces[0].

Device kernel per core (batch shard 1024 rows, 8 blocks of 128):
    load base/source block [128, 4096] f32
    D  = source - base                    (DVE, output bf16)
    Dt = PE-transpose of D in [128,128] chunks (bf16, via identity matmul)
    T^T[k,128] = sum_j W_sel_chunk_j.T @ Dt_chunk_j   (32 bf16 matmuls, psum)
    out_block = base + (T^T).T @ W_selT   (8 fp32 matmuls N=512 + DVE add)
    store out_block

The correction term has rms ~0.25 vs base ~1.0, so bf16 rounding on the
first matmul contributes ~1e-3 absolute error on the output; the second
matmul and the final add are fp32.
"""

import os
import numpy as np
import ml_dtypes

import concourse.bass as bass
import concourse.tile as tile
from concourse import bacc, masks, mybir
from concourse.bass_utils import run_bass_kernel_spmd

N_CORES = 8
B_FULL = 8192
D = 4096
K = 128
BS = B_FULL // N_CORES  # 1024 rows per core
NB = BS // 128          # 8 blocks of 128 rows
NCH = D // 128          # 32 contraction chunks of 128

F32 = mybir.dt.float32
BF16 = mybir.dt.bfloat16


def _build(mm1_dtype="bf16", mm2_f32r=False):
    nc = bacc.Bacc("TRN2", target_bir_lowering=False, debug=False)

    base_d = nc.dram_tensor("base", [BS, D], F32, kind="ExternalInput").ap()
    src_d = nc.dram_tensor("source", [BS, D], F32, kind="ExternalInput").ap()
    w1_dt = BF16 if mm1_dtype == "bf16" else F32
    # fp32r is bit-identical to fp32; declaring the whole w2/ttt path as
    # fp32r satisfies the BIR verifier's "rounded to FP32r" producer rule.
    w2_dt = mybir.dt.float32r if mm2_f32r else F32
    # w1: chunk-major W_sel: w1[p, 128*j + k] = W_sel[128*j + p, k]
    w1_d = nc.dram_tensor("w1", [128, D], w1_dt, kind="ExternalInput").ap()
    # w2: W_sel.T  (k on partitions)
    w2_d = nc.dram_tensor("w2", [K, D], w2_dt, kind="ExternalInput").ap()
    out_d = nc.dram_tensor("out", [BS, D], F32, kind="ExternalOutput").ap()

    with tile.TileContext(nc) as tc:
        with (
            tc.tile_pool(name="wpool", bufs=1) as wpool,
            tc.tile_pool(name="ipool", bufs=1) as ipool,
            tc.tile_pool(name="spool", bufs=3) as spool,
            tc.tile_pool(name="dpool", bufs=2) as dpool,
            tc.tile_pool(name="dtpool", bufs=2) as dtpool,
            tc.tile_pool(name="ttpool", bufs=2) as ttpool,
            tc.tile_pool(name="opool", bufs=4) as opool,
            tc.tile_pool(name="ptr", bufs=2, space="PSUM") as ptrpool,
            tc.tile_pool(name="pT", bufs=2, space="PSUM") as pTpool,
            tc.tile_pool(name="p2", bufs=4, space="PSUM") as p2pool,
        ):
            w1_sb = wpool.tile([128, D], w1_dt, tag="w1")
            nc.sync.dma_start(w1_sb[:], w1_d[:])
            w2_sb = wpool.tile([K, D], w2_dt, tag="w2")
            nc.sync.dma_start(w2_sb[:], w2_d[:])
            ident = ipool.tile([128, 128], w1_dt, tag="ident")
            masks.make_identity(nc, ident[:])

            # transposes per psum bank: bf16 bank holds 8 chunks, f32 bank 4
            per_bank = 8 if w1_dt == BF16 else 4
            bank_free = 128 * per_bank

            # process blocks in pairs so mm1 runs with N=256 moving dim
            for ip in range(NB // 2):
                ots = []
                # Dt for the pair, block-major: dtt[p, D*par + 128*j + b]
                dtt = dtpool.tile([128, 2 * D], w1_dt, tag="dtt")
                for par in range(2):
                    i = 2 * ip + par
                    # base loads straight into the output tile; the
                    # correction is accumulated in place later.
                    ot = opool.tile([128, D], F32, tag="ot")
                    nc.sync.dma_start(ot[:], base_d[128 * i : 128 * (i + 1), :])
                    st = spool.tile([128, D], F32, tag="st")
                    nc.sync.dma_start(st[:], src_d[128 * i : 128 * (i + 1), :])
                    ots.append(ot)

                    if mm1_dtype == "bf16":
                        db = dpool.tile([128, D], BF16, tag="db")
                        nc.vector.tensor_sub(db[:], st[:], ot[:])
                    else:
                        db = st  # subtract in place, keep f32
                        nc.vector.tensor_sub(db[:], st[:], ot[:])

                    for g in range(NCH // per_bank):
                        ps = ptrpool.tile([128, bank_free], w1_dt, tag="ps")
                        for q in range(per_bank):
                            j = per_bank * g + q
                            nc.tensor.transpose(
                                ps[:, 128 * q : 128 * (q + 1)],
                                db[:, 128 * j : 128 * (j + 1)],
                                ident[:],
                            )
                        nc.scalar.copy(
                            dtt[:, D * par + bank_free * g : D * par + bank_free * (g + 1)],
                            ps[:],
                        )

                # mm1: T^T for both blocks, N=256 via 3D AP (par, b) over dtt
                dt3 = dtt[:].rearrange("p (par j b) -> p j par b", par=2, b=128)
                pt = pTpool.tile([K, 256], F32, tag="pt")
                for j in range(NCH):
                    nc.tensor.matmul(
                        pt[:],
                        w1_sb[:, 128 * j : 128 * (j + 1)],
                        dt3[:, j],
                        start=(j == 0),
                        stop=(j == NCH - 1),
                    )
                ttt = ttpool.tile([K, 256], w2_dt, tag="ttt")
                nc.vector.tensor_copy(ttt[:], pt[:])

                for par in range(2):
                    i = 2 * ip + par
                    ot = ots[par]
                    for dj in range(D // 512):
                        p2t = p2pool.tile([128, 512], F32, tag="p2t")
                        lhs = ttt[:, 128 * par : 128 * (par + 1)]
                        rhs = w2_sb[:, 512 * dj : 512 * (dj + 1)]
                        nc.tensor.matmul(p2t[:], lhs, rhs, start=True, stop=True)
                        nc.vector.tensor_add(
                            ot[:, 512 * dj : 512 * (dj + 1)],
                            ot[:, 512 * dj : 512 * (dj + 1)],
                            p2t[:],
                        )
                    nc.sync.dma_start(out_d[128 * i : 128 * (i + 1), :], ot[:])

    nc.compile()
    return nc


_NC_CACHE = {}


def _get_nc(mm1_dtype, mm2_f32r):
    key = (mm1_dtype, mm2_f32r)
    if key not in _NC_CACHE:
        _NC_CACHE[key] = _build(mm1_dtype, mm2_f32r)
    return _NC_CACHE[key]


def make_in_maps(inputs, mm1_dtype="bf16"):
    base = np.ascontiguousarray(np.asarray(inputs["base"], dtype=np.float32))
    source = np.ascontiguousarray(np.asarray(inputs["source"], dtype=np.float32))
    subspaces = np.asarray(inputs["subspaces"])
    W = np.asarray(inputs["W"], dtype=np.float32)
    assert base.shape == (B_FULL, D) and source.shape == (B_FULL, D)

    sel = np.asarray(subspaces[0]).astype(np.int64)  # shared index set
    W_sel = np.ascontiguousarray(W[:, sel])          # [D, K] f32
    # chunk-major layout: w1[p, 128*j + k] = W_sel[128*j + p, k]
    w1 = np.ascontiguousarray(
        W_sel.reshape(NCH, 128, K).transpose(1, 0, 2).reshape(128, D)
    )
    if mm1_dtype == "bf16":
        w1 = w1.astype(ml_dtypes.bfloat16)
    w2 = np.ascontiguousarray(W_sel.T)               # [K, D] f32

    in_maps = []
    for c in range(N_CORES):
        in_maps.append(
            {
                "base": np.ascontiguousarray(base[c * BS : (c + 1) * BS]),
                "source": np.ascontiguousarray(source[c * BS : (c + 1) * BS]),
                "w1": w1,
                "w2": w2,
            }
        )
    return in_maps


def run(inputs, trace=False, mm1_dtype="bf16", mm2_f32r=False):
    nc = _get_nc(mm1_dtype, mm2_f32r)
    in_maps = make_in_maps(inputs, mm1_dtype)
    res = run_bass_kernel_spmd(nc, in_maps, list(range(N_CORES)), trace=trace)
    out = np.concatenate([r["out"] for r in res.results], axis=0)
    return out, res


def kernel(**inputs) -> np.ndarray:
    mm1_dtype = os.environ.get("LRI_MM1", "bf16")
    mm2_f32r = os.environ.get("LRI_MM2_F32R", "0") == "1"
    out, _ = run(inputs, trace=False, mm1_dtype=mm1_dtype, mm2_f32r=mm2_f32r)
    return out
